# revision 1
# baseline (speedup 1.0000x reference)
"""Trainium2 Bass kernel for nn_AutoregressiveLSA — bf16 fused pipeline.

Reference math (complex, per batch b):
    Q  = WKQ @ E                      [2d, T]
    S  = E^H @ Q, keep i <= j         [T, T]
    out= WPV @ (E @ S) / rho_j        [d, T], cols 1..T-2 returned

Re-associated as out = (WPV @ E) @ S, computed transposed:
    PT[t, d]  = (WPV @ E)^T           (lhsT = E, rhs = WPV^T)
    outT[j,d] = sum_{i<=j} S[i,j] PT[i,d] / rho_j

Sharding: data-parallel over batch, one NeuronCore per batch element.

All matmuls run in bf16 (1 col/cycle on the PE at any free width, same
throughput as f32r but half the DMA/SBUF footprint); accumulation stays
f32 in PSUM. End-to-end bf16 numerics land at ~7e-3 relative error vs
the 2e-2 gate.

Structure (single pass per phase, E loaded once and kept resident):
  A1: Q = WKQ @ E, m-major with streamed WKQ^T blocks. Karatsuba
      M1=Wr.Er M2=Wi.Ei M3=(Wr-Wi)(Er-Ei); Qr=M1-M2, Qi=M1+M2-M3.
      Q round-trips DRAM in bf16 (the only intermediate that does).
  A2: PT = E^T @ WPV^T with the same Karatsuba shape; PT (re, im, sum)
      stays SBUF-resident for phase BC.
  BC: fused score+output, one 128-col output block at a time (Q staged
      in 256-col panels). S-block row ib is matmul'd (Karatsuba
      M1=Er.Qr M2=Ei.Qi M3=(Er-Ei)(Qr+Qi); Sr=M1+M2, Si=M3-M1+M2),
      evacuated to a bf16 row store, and contracted with PT into 3
      pinned PSUM accumulator banks (Karatsuba N1=Sr.PTr N2=Si.PTi
      N3=(Sr+Si)(PTr+PTi); Ore=N1-N2, Oim=N3-N1-N2, scaled by 1/rho on
      the Act engine). The contraction trails the S matmuls by 3 rows
      (woven across block boundaries) so PSUM evacuation latency never
      stalls the PE. S never touches DRAM. PSUM: 5-bank rotation for
      the S products + 3 pinned accumulator banks.
"""

import numpy as np
import ml_dtypes

import concourse.bass as bass
import concourse.mybir as mybir
import concourse.tile as tile
from concourse import bacc
from concourse.bass_utils import run_bass_kernel_spmd

F32 = mybir.dt.float32
BF16 = mybir.dt.bfloat16

# Problem dims (hardcoded per contract)
B = 8
D2 = 1024   # 2*dim, channel dim of E
T = 2048    # sequence length
D = 512     # output channel dim
P = 128
KC = D2 // P   # k-tiles over channel dim
MB = D2 // P   # m-tiles for Q rows
TB = T // P    # 128-blocks over sequence
A1P = 512      # A1 column-panel width
NJP = T // A1P
SPAN = 256     # BC column-panel width
NSP = T // SPAN


def _mm(nc, out, lhsT, rhs, start, stop):
    nc.tensor.matmul(out, lhsT, rhs, start=start, stop=stop)


def build_module():
    nc = bacc.Bacc(target_bir_lowering=False, trn_type="TRN2")

    e_re = nc.dram_tensor("e_re", [D2, T], BF16, kind="ExternalInput")
    e_im = nc.dram_tensor("e_im", [D2, T], BF16, kind="ExternalInput")
    wt_re = nc.dram_tensor("wt_re", [MB, P, KC, P], BF16, kind="ExternalInput")
    wt_im = nc.dram_tensor("wt_im", [MB, P, KC, P], BF16, kind="ExternalInput")
    wt_df = nc.dram_tensor("wt_df", [MB, P, KC, P], BF16, kind="ExternalInput")
    wv_re = nc.dram_tensor("wv_re", [D2, D], BF16, kind="ExternalInput")
    wv_im = nc.dram_tensor("wv_im", [D2, D], BF16, kind="ExternalInput")
    wv_df = nc.dram_tensor("wv_df", [D2, D], BF16, kind="ExternalInput")
    trimask = nc.dram_tensor("trimask", [P, P], BF16, kind="ExternalInput")
    rho = nc.dram_tensor("rho", [P, TB], F32, kind="ExternalInput")
    outT_re = nc.dram_tensor("outT_re", [T, D], F32, kind="ExternalOutput")
    outT_im = nc.dram_tensor("outT_im", [T, D], F32, kind="ExternalOutput")

    with tile.TileContext(nc) as tc:
        with tc.tile_pool(name="dram", bufs=1, space="DRAM") as dram, \
             tc.tile_pool(name="ps", bufs=1, space="PSUM") as ps, \
             tc.tile_pool(name="eres", bufs=1) as eres, \
             tc.tile_pool(name="ptres", bufs=1) as ptres, \
             tc.tile_pool(name="cst", bufs=1) as cst:
            q_re = dram.tile([P, MB, T], BF16, tag="q_re")
            q_im = dram.tile([P, MB, T], BF16, tag="q_im")

            er = eres.tile([P, KC, T], BF16, tag="er")
            ei = eres.tile([P, KC, T], BF16, tag="ei")
            ed = eres.tile([P, KC, T], BF16, tag="ed")
            ptr = ptres.tile([P, TB, D], BF16, tag="ptr")
            pti = ptres.tile([P, TB, D], BF16, tag="pti")
            pts = ptres.tile([P, TB, D], BF16, tag="pts")
            mask_sb = cst.tile([P, P], BF16, tag="mask")
            rho_sb = cst.tile([P, TB], F32, tag="rho")
            # panel-0/1 Q tiles for phase BC, reserved outside the phase
            # pools so their gather DMAs can run during A2
            qr_e0 = cst.tile([P, MB, SPAN], BF16, tag="qr_e0")
            qi_e0 = cst.tile([P, MB, SPAN], BF16, tag="qi_e0")
            qs_e0 = cst.tile([P, MB, SPAN], BF16, tag="qs_e0")
            qr_e1 = cst.tile([P, MB, SPAN], BF16, tag="qr_e1")
            qi_e1 = cst.tile([P, MB, SPAN], BF16, tag="qi_e1")
            # PE warm-up operand: keeps the tensor engine busy (and its
            # p-state ramped) while the first real operands stream in
            wu = cst.tile([P, 512], BF16, tag="wu")

            # PSUM: 8 full banks p0..p7 (bufs=1). A1/A2 rotate 3-bank sets
            # over all 8; BC uses p0..p4 for the S rotation (+ even-block
            # reuse) and pins p5..p7 for the odd output block accumulators.
            _rr = [0]

            def psum_set(width, nbanks=8, base=0, count=3):
                i = _rr[0]
                _rr[0] += count
                out = []
                for k in range(count):
                    t = base + (i + k) % nbanks
                    out.append(ps.tile([P, 512], F32, tag=f"p{t}",
                                       name=f"ps{t}_{i}_{k}")[:, :width])
                return out

            def load_e_dma(c, ei_queue=None):
                # DMA transfers drain through one global FIFO ordered by
                # issue-readiness: chunk 0's ei goes on the Pool queue (so it
                # lands right after er0), later ei's go on sync AFTER the wt
                # blocks so weights aren't pushed behind bulk E traffic
                cs = bass.ds(c * A1P, A1P)
                nc.sync.dma_start(
                    er[:, :, cs],
                    e_re[:, cs].rearrange("(kc p) t -> p kc t", p=P))
                q = ei_queue or nc.gpsimd
                q.dma_start(
                    ei[:, :, cs],
                    e_im[:, cs].rearrange("(kc p) t -> p kc t", p=P))

            def ed_sub(c):
                # issued only when the chunk-c DMAs are already in flight a
                # full iteration ahead, so this never blocks the DVE queue
                cs = bass.ds(c * A1P, A1P)
                nc.vector.tensor_sub(ed[:, :, cs], er[:, :, cs], ei[:, :, cs])

            # ---- Phases A1 + A2 ----
            # wvp is opened around both so the wv loads (A2's inputs) can be
            # issued up front and stream in during A1.
            with tc.tile_pool(name="wvp", bufs=1) as wvp:
                wvr = wvp.tile([P, KC, D], BF16, tag="wvr")
                wvi = wvp.tile([P, KC, D], BF16, tag="wvi")
                wvd = wvp.tile([P, KC, D], BF16, tag="wvd")

                # ---- Phase A1: Q = WKQ @ E -> DRAM (bf16) ----
                # m-major: each WKQ^T m-block streamed once; E chunks load
                # on the first m-sweep and stay resident.
                with tc.tile_pool(name="wtp", bufs=2) as wtp, \
                     tc.tile_pool(name="qev", bufs=3) as qev:
                    nc.vector.memzero(wu[:])
                    wt_t = {}

                    def load_wt(m):
                        tr = wtp.tile([P, KC, P], BF16, tag="wtr", name=f"wtr{m}")
                        ti = wtp.tile([P, KC, P], BF16, tag="wti", name=f"wti{m}")
                        td = wtp.tile([P, KC, P], BF16, tag="wtd", name=f"wtd{m}")
                        nc.sync.dma_start(tr[:], wt_re[m])
                        nc.sync.dma_start(ti[:], wt_im[m])
                        nc.sync.dma_start(td[:], wt_df[m])
                        wt_t[m] = (tr, ti, td)

                    load_e_dma(0)
                    load_wt(0)
                    load_wt(1)
                    for c in range(1, NJP):
                        load_e_dma(c, ei_queue=nc.sync)
                    nc.gpsimd.dma_start(mask_sb[:], trimask[:])
                    nc.gpsimd.dma_start(rho_sb[:], rho[:])
                    for _ in range(11):
                        (pw,) = psum_set(512, count=1)
                        _mm(nc, pw, wu[:, :P], wu[:], True, True)
                    ed_sub(0)
                    ed_sub(1)

                    def a1_set(m, jp):
                        js = bass.ds(jp * A1P, A1P)
                        tr, ti, td = wt_t[m]
                        pa, pb, pc = psum_set(A1P)
                        # product-major: pa's mms need only er, pb's only
                        # ei, pc's only ed — smooths cold-start arrival
                        for dst, lh, rh in ((pa, tr, er), (pb, ti, ei), (pc, td, ed)):
                            for kc in range(KC):
                                _mm(nc, dst, lh[:, kc], rh[:, kc, js],
                                    kc == 0, kc == KC - 1)
                        qr_sb = qev.tile([P, A1P], BF16, tag="qr", name=f"qr{m}_{jp}")
                        qi_sb = qev.tile([P, A1P], BF16, tag="qi", name=f"qi{m}_{jp}")
                        nc.scalar.copy(qr_sb[:], pa[:])
                        nc.vector.tensor_sub(qr_sb[:], qr_sb[:], pb[:])
                        nc.scalar.copy(qi_sb[:], pa[:])
                        nc.vector.tensor_add(qi_sb[:], qi_sb[:], pb[:])
                        nc.vector.tensor_sub(qi_sb[:], qi_sb[:], pc[:])
                        nc.gpsimd.dma_start(q_re[:, m, js], qr_sb[:])
                        nc.gpsimd.dma_start(q_im[:, m, js], qi_sb[:])

                    # m=0 and m=1 interleaved jp-major: halves the E-chunk
                    # consumption rate so the (globally serialized) DMA
                    # stream stays ahead during the cold start
                    for jp in range(NJP):
                        if jp >= 1:
                            if jp + 1 < NJP:
                                ed_sub(jp + 1)
                            if jp == NJP - 1:
                                load_wt(2)  # wt0's buffer frees after (0, jp3)
                        a1_set(0, jp)
                        a1_set(1, jp)
                    wt_t.pop(0), wt_t.pop(1)
                    for m in range(2, MB):
                        if m + 1 < MB:
                            load_wt(m + 1)
                        if m == 2:
                            nc.sync.dma_start(wvr[:], wv_re[:].rearrange("(kc p) d -> p kc d", p=P))
                            nc.sync.dma_start(wvi[:], wv_im[:].rearrange("(kc p) d -> p kc d", p=P))
                            nc.sync.dma_start(wvd[:], wv_df[:].rearrange("(kc p) d -> p kc d", p=P))
                        for jp in range(NJP):
                            a1_set(m, jp)
                        wt_t.pop(m)

                # panel-0/1 Q gathers and the panel-0 sum run during A2
                js0 = bass.ds(0, SPAN)
                js1 = bass.ds(SPAN, SPAN)
                nc.sync.dma_start(qr_e0[:], q_re[:, :, js0])
                nc.sync.dma_start(qi_e0[:], q_im[:, :, js0])
                nc.sync.dma_start(qr_e1[:], q_re[:, :, js1])
                nc.sync.dma_start(qi_e1[:], q_im[:, :, js1])
                nc.vector.tensor_add(qs_e0[:], qr_e0[:], qi_e0[:])

                # ---- Phase A2: PT = (WPV @ E)^T -> SBUF resident (bf16) ----
                for tb in range(TB):
                    ts_ = bass.ts(tb, P)
                    pa, pb, pc = psum_set(D)
                    for kc in range(KC):
                        first, last = kc == 0, kc == KC - 1
                        _mm(nc, pa, er[:, kc, ts_], wvr[:, kc], first, last)
                        _mm(nc, pb, ei[:, kc, ts_], wvi[:, kc], first, last)
                        _mm(nc, pc, ed[:, kc, ts_], wvd[:, kc], first, last)
                    nc.scalar.copy(ptr[:, tb], pa[:])
                    nc.vector.tensor_sub(ptr[:, tb], ptr[:, tb], pb[:])
                    nc.scalar.copy(pti[:, tb], pa[:])
                    nc.vector.tensor_add(pti[:, tb], pti[:, tb], pb[:])
                    nc.vector.tensor_sub(pti[:, tb], pti[:, tb], pc[:])
                    nc.vector.tensor_add(pts[:, tb], ptr[:, tb], pti[:, tb])

            # ---- Phase BC: fused S panels + output contraction ----
            # Panel sp covers j in [256sp, 256sp+256) = blocks jb0=2sp,
            # jb1=2sp+1. Rows ib=0..jb1 (last row right-half only).
            with tc.tile_pool(name="qpp", bufs=2) as qpp, \
                 tc.tile_pool(name="sst", bufs=1) as sst, \
                 tc.tile_pool(name="oev", bufs=1) as oev:
                srs = sst.tile([P, TB, P], BF16, tag="srs")
                sis = sst.tile([P, TB, P], BF16, tag="sis")
                sss = sst.tile([P, TB, P], BF16, tag="sss")

                qpan = {}

                def load_qpan_dma(sp):
                    js = bass.ds(sp * SPAN, SPAN)
                    qr_p = qpp.tile([P, MB, SPAN], BF16, tag="qr_p", name=f"qrp{sp}")
                    qi_p = qpp.tile([P, MB, SPAN], BF16, tag="qi_p", name=f"qip{sp}")
                    qs_p = qpp.tile([P, MB, SPAN], BF16, tag="qs_p", name=f"qsp{sp}")
                    nc.sync.dma_start(qr_p[:], q_re[:, :, js])
                    nc.sync.dma_start(qi_p[:], q_im[:, :, js])
                    qpan[sp] = (qr_p, qi_p, qs_p)

                def qs_add(sp):
                    # DMAs for sp were issued at least a panel ago: no block
                    qr_p, qi_p, qs_p = qpan[sp]
                    nc.vector.tensor_add(qs_p[:], qr_p[:], qi_p[:])

                # panels 0/1 were gathered during A2; panel 1's sum tile
                # comes from qpp (its add runs right at BC start)
                qs_p1 = qpp.tile([P, MB, SPAN], BF16, tag="qs_p", name="qsp1")
                qpan[0] = (qr_e0, qi_e0, qs_e0)
                qpan[1] = (qr_e1, qi_e1, qs_p1)
                qs_add(1)
                # land the first S-product set on banks A2 released earliest
                _rr[0] += (1 - _rr[0]) % 5

                # One j-block (128 cols) at a time; Q staged in 256-wide
                # panels (jb//2). The contraction of S row ib trails the
                # S matmuls by 2 rows (global order across blocks) so the
                # PSUM-evacuation chain never stalls the PE. Accumulators
                # for block jb live in pinned banks p5..p7; their evacuation
                # overlaps the next block's first S rows.
                pend = []      # queued (jb, ib) contractions, 2-row lag
                evac_pend = []  # accumulator handles awaiting output evac

                def c_set(jb, ib, acc):
                    # n3's bank is freed first by the previous block's
                    # output evacuation, so start there
                    first, last = ib == 0, ib == jb
                    _mm(nc, acc[2], sss[:, ib], pts[:, ib], first, last)
                    _mm(nc, acc[0], srs[:, ib], ptr[:, ib], first, last)
                    _mm(nc, acc[1], sis[:, ib], pti[:, ib], first, last)

                def out_evac(jb, acc):
                    n1, n2, n3 = acc
                    jbs = bass.ts(jb, P)
                    t_re = oev.tile([P, D], F32, tag="t_re", name=f"tre{jb}")
                    t_im = oev.tile([P, D], F32, tag="t_im", name=f"tim{jb}")
                    rho_ap = rho_sb[:, jb:jb + 1]
                    if jb == TB - 1:
                        # final block is on the critical path to kernel end:
                        # pipeline DVE -> Act -> DMA in 256-col halves
                        for h in (bass.ds(0, 256), bass.ds(256, 256)):
                            nc.scalar.copy(t_re[:, h], n1[:, h])
                            nc.vector.tensor_sub(t_re[:, h], t_re[:, h], n2[:, h])
                            nc.scalar.copy(t_im[:, h], n3[:, h])
                            nc.vector.tensor_sub(t_im[:, h], t_im[:, h], n1[:, h])
                            nc.vector.tensor_sub(t_im[:, h], t_im[:, h], n2[:, h])
                            nc.scalar.mul(t_re[:, h], t_re[:, h], rho_ap)
                            nc.scalar.mul(t_im[:, h], t_im[:, h], rho_ap)
                        nc.sync.dma_start(outT_re[jbs, :], t_re[:])
                        nc.sync.dma_start(outT_im[jbs, :], t_im[:])
                        return
                    nc.scalar.copy(t_im[:], n3[:])
                    nc.scalar.copy(t_re[:], n1[:])
                    nc.vector.tensor_sub(t_im[:], t_im[:], n1[:])
                    nc.vector.tensor_sub(t_re[:], t_re[:], n2[:])
                    nc.vector.tensor_sub(t_im[:], t_im[:], n2[:])
                    nc.scalar.mul(t_re[:], t_re[:], rho_ap)
                    nc.scalar.mul(t_im[:], t_im[:], rho_ap)
                    nc.gpsimd.dma_start(outT_re[jbs, :], t_re[:])
                    nc.gpsimd.dma_start(outT_im[jbs, :], t_im[:])

                def drain_c(keep, upto=None):
                    # upto=(jb, ib): also pop older-block entries whose S row
                    # slot is about to be overwritten by row ib's evacuation
                    while len(pend) > keep or (
                            upto is not None and pend
                            and pend[0][0] < upto[0] and pend[0][1] <= upto[1]):
                        jbq, ibq, accq = pend.pop(0)
                        c_set(jbq, ibq, accq)
                        if ibq == jbq:
                            evac_pend.append((jbq, accq))
                    while evac_pend:
                        out_evac(*evac_pend.pop(0))

                for jb in range(TB):
                    sp = jb // 2
                    half = bass.ds((jb % 2) * P, P)
                    if jb % 2 == 0:
                        if jb > 0 and sp + 1 < NSP:
                            load_qpan_dma(sp + 1)
                    else:
                        if sp >= 1 and sp + 1 < NSP:
                            qs_add(sp + 1)
                    qr_p, qi_p, qs_p = qpan[sp]
                    if jb % 2 == 1:
                        del qpan[sp]
                    acc = (ps.tile([P, 512], F32, tag="p5", name=f"n1_{jb}"),
                           ps.tile([P, 512], F32, tag="p6", name=f"n2_{jb}"),
                           ps.tile([P, 512], F32, tag="p7", name=f"n3_{jb}"))
                    for ib in range(jb + 1):
                        ibs = bass.ts(ib, P)
                        pa, pb, pc = psum_set(P, nbanks=5)
                        for dst, lh, rh in ((pa, er, qr_p), (pb, ei, qi_p),
                                            (pc, ed, qs_p)):
                            for kc in range(KC):
                                _mm(nc, dst, lh[:, kc, ibs], rh[:, kc, half],
                                    kc == 0, kc == KC - 1)
                        keep = 3 if jb < 8 else 4
                        drain_c(keep, upto=(jb, ib))
                        nc.scalar.copy(srs[:, ib], pa[:])
                        nc.vector.tensor_add(srs[:, ib], srs[:, ib], pb[:])
                        nc.scalar.copy(sis[:, ib], pc[:])
                        nc.vector.tensor_sub(sis[:, ib], sis[:, ib], pa[:])
                        nc.vector.tensor_add(sis[:, ib], sis[:, ib], pb[:])
                        if ib == jb:   # diagonal block: causal mask
                            nc.vector.tensor_mul(srs[:, ib], srs[:, ib], mask_sb[:])
                            nc.vector.tensor_mul(sis[:, ib], sis[:, ib], mask_sb[:])
                        nc.vector.tensor_add(sss[:, ib], srs[:, ib], sis[:, ib])
                        pend.append((jb, ib, acc))
                drain_c(0)

    nc.compile()
    return nc


_NC_CACHE = None


def _get_module():
    global _NC_CACHE
    if _NC_CACHE is None:
        _NC_CACHE = build_module()
    return _NC_CACHE


def prep_shared(WKQ_re, WKQ_im, WPV_re, WPV_im):
    """Host-side weight prep, shared across cores (all bf16)."""
    bf = ml_dtypes.bfloat16

    def blk(w):  # WKQ^T blocked for per-m lhsT streaming
        wt = np.ascontiguousarray(w.T)            # [c, c']
        return np.ascontiguousarray(
            wt.reshape(KC, P, MB, P).transpose(2, 1, 0, 3)).astype(bf)

    shared = {
        "wt_re": blk(WKQ_re),
        "wt_im": blk(WKQ_im),
        "wt_df": blk(WKQ_re - WKQ_im),
        "wv_re": np.ascontiguousarray(WPV_re.T).astype(bf),
        "wv_im": np.ascontiguousarray(WPV_im.T).astype(bf),
        "wv_df": np.ascontiguousarray((WPV_re - WPV_im).T).astype(bf),
        "trimask": np.triu(np.ones((P, P), np.float32)).astype(bf),
    }
    j = np.arange(T, dtype=np.float32)
    rho = 1.0 / np.maximum(j, 1.0)
    shared["rho"] = np.ascontiguousarray(rho.reshape(TB, P).T)  # [p, jb]
    return shared


def kernel(E_re, E_im, WKQ_re, WKQ_im, WPV_re, WPV_im):
    bf = ml_dtypes.bfloat16
    E_re = np.asarray(E_re, dtype=np.float32)
    E_im = np.asarray(E_im, dtype=np.float32)
    shared = prep_shared(np.asarray(WKQ_re, np.float32),
                         np.asarray(WKQ_im, np.float32),
                         np.asarray(WPV_re, np.float32),
                         np.asarray(WPV_im, np.float32))
    in_maps = []
    for b in range(B):
        m = dict(shared)
        m["e_re"] = np.ascontiguousarray(E_re[b]).astype(bf)
        m["e_im"] = np.ascontiguousarray(E_im[b]).astype(bf)
        in_maps.append(m)

    nc = _get_module()
    res = run_bass_kernel_spmd(nc, in_maps, core_ids=list(range(B)))

    out = np.empty((B, D, T - 2), dtype=np.complex64)
    for b in range(B):
        r = res.results[b]["outT_re"]  # [T, D]
        i = res.results[b]["outT_im"]
        full = (r + 1j * i.astype(np.complex64)).T  # [D, T]
        out[b] = full[:, 1 : T - 1]
    return out

